# revision 1
# baseline (speedup 1.0000x reference)
"""Trainium2 Bass kernel for a dense transformer DecoderLayer.

Layer: x = q
  x += SelfAttn(LN1(x))   (causal, 8 heads)
  x += CrossAttn(LN2(x), k, v)
  x += FFN(LN3(x))        (E -> 4E relu -> E)

Sharding: 8 cores = (batch b = core//2, parity p = core%2). Core (b, p)
owns the 8 odd-or-even 128-row stripes of batch b's 2048 query rows.
Host permutes q[b]^T columns to [partner stripes | own stripes] so the
device program is identical on every core (SPMD); the parity-dependent
causal boundary is carried by a data mask (mp = all-ones or all-zeros).

Device dataflow is fully "transposed": the residual stream lives as
x^T [E=512 partitions(4 tiles), tokens] so no on-device activation
transposes are needed except the final 128x128 PE transposes on output.
Scores are computed transposed (S^T [s,q]) so the softmax denominator
falls out of the P@V matmul via an appended ones-column on V.
"""

import numpy as np
import ml_dtypes

import concourse.bass as bass
import concourse.tile as tile
from concourse import bacc
from concourse import mybir
from concourse.bass_utils import run_bass_kernel_spmd

F32 = mybir.dt.float32
F32R = mybir.dt.float32r
BF16 = mybir.dt.bfloat16

B, T_FULL, E, H, D, FW = 4, 2048, 512, 8, 64, 4
EC = E // 128           # e-chunks
F = FW * E              # ffn hidden
FC = F // 128
EPS = 1e-5
NCORES = 8


def _pieces(a, b, step=512):
    """Split [a, b) at multiples of `step` (PSUM-bank aligned pieces)."""
    out = []
    while a < b:
        nxt = min(b, (a // step + 1) * step)
        out.append((a, nxt))
        a = nxt
    return out


def _pieces_bf(a, b):
    """Matmul output pieces: one PSUM bank (512 f32) per matmul — walrus
    rejects bank-spanning matmul outputs."""
    return _pieces(a, b, 512)


def build_nc(T=T_FULL):
    R = T // 2           # own query columns (packed at [R:T])
    G = R // 128         # own 128-col groups
    NCH = T // 128       # total s-chunks

    nc = bacc.Bacc(None, target_bir_lowering=False)

    # ---------------- DRAM I/O ----------------
    qTp = nc.dram_tensor("qTp", [E, T], F32, kind="ExternalInput")
    kT = nc.dram_tensor("kT", [E, T], BF16, kind="ExternalInput")
    vT = nc.dram_tensor("vT", [E, T], BF16, kind="ExternalInput")
    w = {}
    for nm in ("wq_s", "wk_s", "wv_s", "wq_c", "wk_c", "wv_c"):
        w[nm] = nc.dram_tensor(nm, [E, H * D], BF16, kind="ExternalInput")
    w["wp_s"] = nc.dram_tensor("wp_s", [H * D, E], BF16, kind="ExternalInput")
    w["wp_c"] = nc.dram_tensor("wp_c", [H * D, E], BF16, kind="ExternalInput")
    w["w1"] = nc.dram_tensor("w1", [E, F], BF16, kind="ExternalInput")
    w["w2"] = nc.dram_tensor("w2", [F, E], BF16, kind="ExternalInput")
    bias_d = {}
    for nm, sz in (("bq_s", H * D), ("bk_s", H * D), ("bq_c", H * D), ("bk_c", H * D),
                   ("bp_s", E), ("bp_c", E), ("b1f", F), ("b2f", E)):
        bias_d[nm] = nc.dram_tensor(nm, [sz], F32, kind="ExternalInput")
    mp_d = nc.dram_tensor("mp", [128, 128], BF16, kind="ExternalInput")
    mtri_d = nc.dram_tensor("mtri", [128, 128], BF16, kind="ExternalInput")
    ident_d = nc.dram_tensor("ident", [128, 128], F32, kind="ExternalInput")
    out_d = nc.dram_tensor("out", [R, E], F32, kind="ExternalOutput")

    with tile.TileContext(nc) as tc:
        with (
            tc.tile_pool(name="resident", bufs=1) as res,
            tc.tile_pool(name="consts", bufs=1) as cpool,
            tc.tile_pool(name="work", bufs=2) as work,
            tc.tile_pool(name="es_pool", bufs=3) as es_pool,
            tc.tile_pool(name="stat", bufs=2) as stat,
            tc.tile_pool(name="drampool", bufs=2, space="DRAM") as drampool,
            tc.tile_pool(name="psum", bufs=1, space="PSUM") as psum,
        ):
            # ---- unified PSUM tags: "sc" 2 banks x2, "o" 2 banks x2 = 8 banks
            PSW = max(512, R)
            def ps_sc(name):
                return psum.tile([128, PSW], F32, name=name, tag="sc", bufs=2)

            def ps_o(name, shape=None):
                return psum.tile(shape or [128, PSW], F32, name=name, tag="o", bufs=2)

            # ---------------- resident loads ----------------
            xto = []        # own-half residual stream [E, R], lives whole kernel
            for c in range(EC):
                t_ = res.tile([128, R], F32, name=f"xto{c}")
                nc.sync.dma_start(
                    out=t_, in_=qTp.rearrange("(c p) t -> c p t", p=128)[c][:, R:T])
                xto.append(t_)

            bias_sb = {}
            for nm in bias_d:
                src = bias_d[nm]
                t_ = cpool.tile([128, src.shape[0] // 128], F32, name=f"b_{nm}")
                nc.sync.dma_start(out=t_, in_=src.rearrange("(c p) -> p c", p=128))
                bias_sb[nm] = t_
            mp_sb = cpool.tile([128, 128], BF16, name="mp_sb")
            nc.sync.dma_start(out=mp_sb, in_=mp_d[:, :])
            mtri_sb = cpool.tile([128, 128], BF16, name="mtri_sb")
            nc.sync.dma_start(out=mtri_sb, in_=mtri_d[:, :])
            ident_sb = cpool.tile([128, 128], F32, name="ident_sb")
            nc.sync.dma_start(out=ident_sb, in_=ident_d[:, :])
            ones_inv_bf = cpool.tile([128, 1], BF16, name="ones_inv_bf")
            nc.vector.memset(ones_inv_bf, 1.0 / E)
            eps_sb = cpool.tile([1, 1], F32, name="eps_sb")
            nc.vector.memset(eps_sb, EPS)

            def load_w(pool, nm, tag):
                src = w[nm]
                if nm.startswith("wp"):
                    t_ = pool.tile([64, H, E], BF16, name=f"sb_{nm}", tag=tag)
                    nc.sync.dma_start(out=t_, in_=src.rearrange("(h d) n -> d h n", h=H))
                else:
                    t_ = pool.tile([128, src.shape[0] // 128, src.shape[1]], BF16,
                                   name=f"sb_{nm}", tag=tag)
                    nc.sync.dma_start(out=t_, in_=src.rearrange("(c p) n -> p c n", p=128))
                return t_

            # ---------------- transposed layernorm ----------------
            def ln_t(xap, t_len, tag, out_pool, out_tags=None, split=None):
                """xap(c, a, b) -> [128, b-a] f32 SBUF AP. Returns 4 bf16
                normalized tiles [128, t_len]."""
                outs = [out_pool.tile([128, t_len], BF16, name=f"xn_{tag}{c}",
                                      tag=(out_tags[c] if out_tags else f"xn_{tag}{c}"))
                        for c in range(EC)]
                pcs = []
                for (a, b_) in _pieces(0, t_len):
                    if split is not None and a < split < b_:
                        pcs += [(a, split), (split, b_)]
                    else:
                        pcs.append((a, b_))
                for (a, b_) in pcs:
                    wd = b_ - a
                    stats = psum.tile([1, 2, 512], F32, name=f"stats_{tag}", tag="o", bufs=2)
                    for c in range(EC):
                        sq = work.tile([128, 512], BF16, name=f"sq_{tag}", tag="lnsq", bufs=2)
                        xb = work.tile([128, 512], BF16, name=f"xb_{tag}", tag="lnxb", bufs=2)
                        xa = xap(c, a, b_)
                        nc.gpsimd.tensor_mul(sq[:, :wd], xa, xa)
                        nc.gpsimd.tensor_copy(xb[:, :wd], xa)
                        nc.tensor.matmul(stats[0:1, 0, :wd], ones_inv_bf, xb[:, :wd],
                                         start=(c == 0), stop=(c == EC - 1))
                        nc.tensor.matmul(stats[0:1, 1, :wd], ones_inv_bf, sq[:, :wd],
                                         start=(c == 0), stop=(c == EC - 1))
                    st_sb = stat.tile([1, 2, 512], F32, name=f"st_{tag}", tag="st_sb")
                    nc.vector.tensor_copy(st_sb[:, :, :wd], stats[:, :, :wd])
                    var = stat.tile([1, 512], F32, name=f"var_{tag}", tag="var", bufs=1)
                    nc.vector.tensor_mul(var[:, :wd], st_sb[:, 0, :wd], st_sb[:, 0, :wd])
                    nc.vector.tensor_sub(var[:, :wd], st_sb[:, 1, :wd], var[:, :wd])
                    nc.scalar.activation(var[:, :wd], var[:, :wd],
                                         mybir.ActivationFunctionType.Ln,
                                         bias=eps_sb[0:1, 0:1])
                    nc.scalar.activation(var[:, :wd], var[:, :wd],
                                         mybir.ActivationFunctionType.Exp, scale=-0.5)
                    mb = work.tile([128, 512], F32, name=f"mb_{tag}", tag="mb")
                    rb = work.tile([128, 512], F32, name=f"rb_{tag}", tag="rb")
                    m_dr = drampool.tile([1, 512], F32, name=f"mdr_{tag}", tag="mdr", bufs=3)
                    r_dr = drampool.tile([1, 512], F32, name=f"rdr_{tag}", tag="rdr", bufs=3)
                    nc.sync.dma_start(out=m_dr[:, :wd], in_=st_sb[:, 0, :wd])
                    nc.sync.dma_start(out=r_dr[:, :wd], in_=var[:, :wd])
                    nc.sync.dma_start(out=mb[:, :wd], in_=m_dr[:, :wd].to_broadcast((128, wd)))
                    nc.sync.dma_start(out=rb[:, :wd], in_=r_dr[:, :wd].to_broadcast((128, wd)))
                    for c in range(EC):
                        tmp = work.tile([128, 512], F32, name=f"lt_{tag}", tag="lntmp", bufs=2)
                        nc.vector.tensor_sub(tmp[:, :wd], xap(c, a, b_), mb[:, :wd])
                        nc.vector.tensor_mul(outs[c][:, a:b_], tmp[:, :wd], rb[:, :wd])
                return outs

            # ---------------- attention building blocks ----------------
            def proj_kt(apool, tags, src_aps, wk_t, bk_sb, tag, step=0):
                """K^T head-pair tiles [128, T] from 4 x [128, T] bf16 APs."""
                ktp = [apool.tile([128, T], BF16, name=f"ktp_{tag}{pp}", tag=tags[pp])
                       for pp in range(4)]
                pcs = _pieces(0, T, step) if step else _pieces_bf(0, T)
                for (a, b_) in pcs:
                    for pp in range(4):
                        ps = ps_sc(f"kps_{tag}")
                        for c in range(EC):
                            nc.tensor.matmul(
                                ps[:, :b_ - a], wk_t[:, c, pp * 128:(pp + 1) * 128],
                                src_aps[c](a, b_),
                                start=(c == 0), stop=(c == EC - 1))
                        nc.vector.tensor_scalar_add(ktp[pp][:, a:b_], ps[:, :b_ - a],
                                                    bk_sb[:, pp:pp + 1])
                return ktp

            def proj_qt(apool, tags, xn_q, wq_t, bq_sb, tag):
                qtp = []
                for pp in range(4):
                    t_ = apool.tile([128, R], BF16, name=f"qtp_{tag}{pp}", tag=tags[pp])
                    for (a, b_) in _pieces_bf(0, R):
                        ps = ps_sc(f"qps_{tag}")
                        for c in range(EC):
                            nc.tensor.matmul(
                                ps[:, :b_ - a], wq_t[:, c, pp * 128:(pp + 1) * 128],
                                xn_q[c][:, a:b_],
                                start=(c == 0), stop=(c == EC - 1))
                        nc.vector.tensor_scalar_add(t_[:, a:b_], ps[:, :b_ - a],
                                                    bq_sb[:, pp:pp + 1])
                    qtp.append(t_)
                return qtp

            def heads(ktp, qtp, v_at, causal, tag):
                """v_at(k) -> lhsT AP [128, D+1] for chunk k. Returns 8 osb."""
                o_out = []
                for h in range(H):
                    pp, hr = h // 2, (h % 2) * 64
                    o_ps = ps_o(f"ops_{tag}", [D + 1, R])
                    if causal:
                        chunk_list = []
                        for g in range(G):
                            chunk_list.append((g, g * 128, mp_sb))
                            chunk_list.append((G + g, g * 128, mtri_sb))
                    else:
                        chunk_list = [(k, 0, None) for k in range(NCH)]
                    first = True
                    for (k, q0, msk) in chunk_list:
                        sc = ps_sc(f"scps_{tag}")
                        for (a, b_) in _pieces_bf(q0, R):
                            nc.tensor.matmul(
                                sc[:, a:b_],
                                ktp[pp][hr:hr + 64, k * 128:(k + 1) * 128],
                                qtp[pp][hr:hr + 64, a:b_],
                                start=True, stop=True)
                        es = es_pool.tile([128, R], BF16, name=f"es_{tag}", tag="es")
                        nc.scalar.activation(es[:, q0:R], sc[:, q0:R],
                                             mybir.ActivationFunctionType.Exp)
                        if msk is not None:
                            nc.vector.tensor_mul(es[:, q0:q0 + 128],
                                                 es[:, q0:q0 + 128], msk)
                        last = (k, q0) == (chunk_list[-1][0], chunk_list[-1][1])
                        for (a, b_) in _pieces_bf(q0, R):
                            nc.tensor.matmul(
                                o_ps[:, a:b_], v_at(k, h), es[:, a:b_],
                                start=first, stop=last and b_ == R,
                                skip_group_check=True)
                        first = False
                    dn = stat.tile([D + 1, R], F32, name=f"dn_{tag}", tag="dn")
                    nc.vector.tensor_copy(dn[D:D + 1, :], o_ps[D:D + 1, :])
                    nc.vector.reciprocal(dn[D:D + 1, :], dn[D:D + 1, :])
                    dn_dr = drampool.tile([1, R], F32, name=f"dndr_{tag}", tag="dndr", bufs=3)
                    nc.sync.dma_start(out=dn_dr, in_=dn[D:D + 1, :])
                    rb_h = work.tile([64, R], F32, name=f"rbh_{tag}", tag="rbh", bufs=2)
                    nc.sync.dma_start(out=rb_h, in_=dn_dr.to_broadcast((64, R)))
                    o_sb = pself.tile([64, R], BF16, name=f"osb_{tag}{h}", tag=f"osb{h}")
                    nc.vector.tensor_mul(o_sb, o_ps[0:D, :], rb_h)
                    o_out.append(o_sb)
                return o_out

            def out_proj_residual(o_list, wp_t, bp_sb, tag):
                for eb in range(EC):
                    ps = ps_o(f"yps_{tag}")
                    for (a, b_) in _pieces_bf(0, R):
                        for h in range(H):
                            nc.tensor.matmul(
                                ps[:, a:b_],
                                wp_t[:, h, eb * 128:(eb + 1) * 128],
                                o_list[h][:, a:b_],
                                start=(h == 0), stop=(h == H - 1))
                    nc.vector.scalar_tensor_tensor(
                        xto[eb], ps[:, :R], bp_sb[:, eb:eb + 1], xto[eb],
                        op0=mybir.AluOpType.add, op1=mybir.AluOpType.add)

            # ================ forward ================
            with tc.tile_pool(name="w_attn", bufs=1) as wat, \
                 tc.tile_pool(name="p_self", bufs=1) as pself:
                # partner-half of residual input (dies after LN1)
                xtp = []
                for c in range(EC):
                    t_ = pself.tile([128, R], F32, name=f"xtp{c}", tag=f"xtp{c}")
                    nc.sync.dma_start(
                        out=t_, in_=qTp.rearrange("(c p) t -> c p t", p=128)[c][:, 0:R])
                    xtp.append(t_)

                wq_t = load_w(wat, "wq_s", "wq")
                wk_t = load_w(wat, "wk_s", "wk")
                wv_t = load_w(wat, "wv_s", "wv")
                wp_t = load_w(wat, "wp_s", "wp")
                wk_ct = load_w(wat, "wk_c", "wk_c")
                wv_ct = load_w(wat, "wv_c", "wv_c")

                def xap1(c, a, b_):
                    if b_ <= R:
                        return xtp[c][:, a:b_]
                    return xto[c][:, a - R:b_ - R]

                xn1 = ln_t(xap1, T, "ln1", pself,
                           out_tags=[f"xn{c}" for c in range(EC)], split=R)

                # --- self-attn projections ---
                ktp_s = proj_kt(pself, ["ktp0", "ktp1", "ktp2", "ktp3"],
                                [lambda a, b_, c=c: xn1[c][:, a:b_] for c in range(EC)],
                                wk_t, bias_sb["bk_s"], "sa")
                qtp_s = proj_qt(pself, ["qtp0", "qtp1", "qtp2", "qtp3"],
                                [x_[:, R:T] for x_ in xn1], wq_t, bias_sb["bq_s"], "sa")
                v_sb_s = []
                for k in range(NCH):
                    ps = ps_sc("vps_sa")
                    for c in range(EC):
                        nc.tensor.matmul(
                            ps[:, :512], xn1[c][:, k * 128:(k + 1) * 128], wv_t[:, c, :],
                            start=(c == 0), stop=(c == EC - 1))
                    vs = pself.tile([128, H, D + 1], BF16, name=f"vsb_sa{k}", tag=f"vsb{k}")
                    nc.vector.tensor_copy(vs[:, :, 0:D],
                                          ps[:, :512].rearrange("p (h d) -> p h d", h=H))
                    nc.gpsimd.memset(vs[:, :, D:D + 1], 1.0)
                    v_sb_s.append(vs)

                # --- cross K/V projections (hoisted; fill PE idle in head phase)
                def kslab(c, a, b_, _cache={}):
                    if a not in _cache:
                        t_ = pself.tile([128, EC, 512], BF16, name="ksl_ca",
                                        tag=f"xtp{(a // 512) % 4}", bufs=1)
                        nc.sync.dma_start(
                            out=t_[:, :, :b_ - a],
                            in_=kT.rearrange("(c p) t -> p c t", p=128)[:, :, a:b_])
                        _cache[a] = t_
                    return _cache[a][:, c, :b_ - a]

                ktp_c = proj_kt(pself, ["xn0", "xn1", "xn2", "xn3"],
                                [lambda a, b_, c=c: kslab(c, a, b_) for c in range(EC)],
                                wk_ct, bias_sb["bk_c"], "ca", step=512)
                v_g = []
                for g2 in range(2):
                    v_g.append(pself.tile([128, 8, H, D + 1], BF16,
                                          name=f"vg{g2}", tag=f"xtp{2 + g2}"))
                for k4 in range(0, NCH, 4):
                    vsl = work.tile([128, EC, 512], BF16, name="vsl_ca", tag="vsl", bufs=2)
                    nc.sync.dma_start(
                        out=vsl,
                        in_=vT.rearrange("(c p) t -> p c t", p=128)[:, :, k4 * 128:(k4 + 4) * 128])
                    for k in range(k4, k4 + 4):
                        off = (k - k4) * 128
                        ps = ps_sc("vps_ca")
                        for c in range(EC):
                            nc.tensor.matmul(
                                ps[:, :512], vsl[:, c, off:off + 128], wv_ct[:, c, :],
                                start=(c == 0), stop=(c == EC - 1))
                        nc.vector.tensor_copy(
                            v_g[k // 8][:, k % 8, :, 0:D],
                            ps[:, :512].rearrange("p (h d) -> p h d", h=H))
                        nc.gpsimd.memset(v_g[k // 8][:, k % 8, :, D:D + 1], 1.0)

                # --- self attention + projection ---
                o1 = heads(ktp_s, qtp_s, lambda k, h: v_sb_s[k][:, h, :], True, "sa")
                out_proj_residual(o1, wp_t, bias_sb["bp_s"], "sa")

                # --- LN2 + cross attention ---
                wq_ct = load_w(wat, "wq_c", "wq")
                wp_ct = load_w(wat, "wp_c", "wp")
                xn2 = ln_t(lambda c, a, b_: xto[c][:, a:b_], R, "ln2", pself,
                           out_tags=["vsb0", "vsb1", "vsb2", "vsb3"])
                qtp_c = proj_qt(pself, ["qtp0", "qtp1", "qtp2", "qtp3"],
                                xn2, wq_ct, bias_sb["bq_c"], "ca")
                o2 = heads(ktp_c, qtp_c, lambda k, h: v_g[k // 8][:, k % 8, h, :], False, "ca")
                out_proj_residual(o2, wp_ct, bias_sb["bp_c"], "ca")

            # ---------------- FFN ----------------
            with tc.tile_pool(name="w_ffn", bufs=1) as wffn, \
                 tc.tile_pool(name="p_ffn", bufs=1) as pffn:
                w1_t = load_w(wffn, "w1", "w1")
                w2_t = load_w(wffn, "w2", "w2")
                xn3 = ln_t(lambda c, a, b_: xto[c][:, a:b_], R, "ln3", pffn)
                h1 = []
                for f in range(FC):
                    ps = ps_sc("hps")
                    for (a, b_) in _pieces_bf(0, R):
                        for c in range(EC):
                            nc.tensor.matmul(
                                ps[:, a:b_], w1_t[:, c, f * 128:(f + 1) * 128],
                                xn3[c][:, a:b_],
                                start=(c == 0), stop=(c == EC - 1))
                    ht = pffn.tile([128, R], BF16, name=f"h1_{f}")
                    nc.vector.tensor_scalar(
                        ht, ps[:, :R], bias_sb["b1f"][:, f:f + 1], 0.0,
                        op0=mybir.AluOpType.add, op1=mybir.AluOpType.max)
                    h1.append(ht)
                for eb in range(EC):
                    ps = ps_o("y2ps")
                    for (a, b_) in _pieces_bf(0, R):
                        for f in range(FC):
                            nc.tensor.matmul(
                                ps[:, a:b_], w2_t[:, f, eb * 128:(eb + 1) * 128],
                                h1[f][:, a:b_],
                                start=(f == 0), stop=(f == FC - 1))
                    nc.vector.scalar_tensor_tensor(
                        xto[eb], ps[:, :R], bias_sb["b2f"][:, eb:eb + 1], xto[eb],
                        op0=mybir.AluOpType.add, op1=mybir.AluOpType.add)

            # ---------------- transpose + store ----------------
            for tb in range(R // 128):
                ot = work.tile([128, E], F32, name="ot", tag="ot", bufs=2)
                for eb in range(EC):
                    ps = psum.tile([128, 128], F32, name="trp", tag="o", bufs=2)
                    nc.tensor.transpose(
                        ps, xto[eb][:, tb * 128:(tb + 1) * 128], ident_sb)
                    nc.vector.tensor_copy(ot[:, eb * 128:(eb + 1) * 128], ps)
                nc.sync.dma_start(out=out_d[tb * 128:(tb + 1) * 128, :], in_=ot)

    nc.compile()
    return nc
# ---------------------------------------------------------------------------
# host side
# ---------------------------------------------------------------------------

_CACHE = {}


def _host_prep(inputs, T=T_FULL):
    ii = {k: np.asarray(v, dtype=np.float32) for k, v in inputs.items()}
    g1, be1, g2, be2, g3, be3 = (ii[k] for k in ("g1", "be1", "g2", "be2", "g3", "be3"))

    def fold_qkv(wstk, g, be, scale=1.0):
        wall = np.transpose(wstk, (1, 0, 2)).reshape(E, H * D)  # [E, H*D]
        return ((g[:, None] * wall) * scale).astype(ml_dtypes.bfloat16), \
               ((be @ wall) * scale).astype(np.float32)

    sc = float(D) ** -0.5
    wq_s, bq_s = fold_qkv(ii["Wq_s"], g1, be1, sc)
    wk_s, bk_s = fold_qkv(ii["Wk_s"], g1, be1)
    wv_s, bv_s = fold_qkv(ii["Wv_s"], g1, be1)
    wq_c, bq_c = fold_qkv(ii["Wq_c"], g2, be2, sc)
    wk_c, bk_c = fold_qkv(ii["Wk_c"], np.ones(E, np.float32), np.zeros(E, np.float32))
    wv_c, bv_c = fold_qkv(ii["Wv_c"], np.ones(E, np.float32), np.zeros(E, np.float32))
    assert np.allclose(bv_s, 0, atol=1e-6) and np.allclose(bv_c, 0, atol=1e-6), \
        "V-projection bias folding not implemented (be nonzero)"
    w1 = (g3[:, None] * ii["W1"]).astype(ml_dtypes.bfloat16)
    b1f = (be3 @ ii["W1"] + ii["b1"]).astype(np.float32)
    w2 = ii["W2"].astype(ml_dtypes.bfloat16)

    shared = dict(
        wq_s=wq_s, wk_s=wk_s, wv_s=wv_s, wp_s=ii["Wp_s"].astype(ml_dtypes.bfloat16),
        wq_c=wq_c, wk_c=wk_c, wv_c=wv_c, wp_c=ii["Wp_c"].astype(ml_dtypes.bfloat16),
        w1=w1, w2=w2, b1f=b1f, b2f=ii["b2"].astype(np.float32),
        bq_s=bq_s, bk_s=bk_s, bq_c=bq_c, bk_c=bk_c,
        bp_s=ii["bp_s"].astype(np.float32), bp_c=ii["bp_c"].astype(np.float32),
        ident=np.eye(128, dtype=np.float32),
        mtri=np.triu(np.ones((128, 128))).astype(ml_dtypes.bfloat16),
    )

    q, k, v = ii["q"], ii["k"], ii["v"]
    n_b = q.shape[0]
    n_stripes = T // 128
    in_maps = []
    for core in range(2 * n_b):
        b, p = core // 2, core % 2
        order = [2 * i + (1 - p) for i in range(n_stripes // 2)] + \
                [2 * i + p for i in range(n_stripes // 2)]
        cols = np.concatenate([np.arange(s * 128, (s + 1) * 128) for s in order])
        m = dict(shared)
        m["qTp"] = np.ascontiguousarray(q[b].T[:, cols])
        m["kT"] = np.ascontiguousarray(k[b].T).astype(ml_dtypes.bfloat16)
        m["vT"] = np.ascontiguousarray(v[b].T).astype(ml_dtypes.bfloat16)
        m["mp"] = np.full((128, 128), float(p), dtype=ml_dtypes.bfloat16)
        in_maps.append(m)
    return in_maps


def _gather(results, T=T_FULL, n_b=B):
    out = np.zeros((n_b, T, E), dtype=np.float32)
    for core in range(2 * n_b):
        b, p = core // 2, core % 2
        r = results[core]["out"].reshape(T // 256, 128, E)
        for j in range(T // 256):
            out[b, (2 * j + p) * 128:(2 * j + p + 1) * 128, :] = r[j]
    return out


def kernel(**inputs):
    if "nc" not in _CACHE:
        _CACHE["nc"] = build_nc(T_FULL)
    nc = _CACHE["nc"]
    in_maps = _host_prep(inputs, T_FULL)
    res = run_bass_kernel_spmd(nc, in_maps, core_ids=list(range(NCORES)))
    return _gather(res.results, T_FULL)

